# revision 26
# baseline (speedup 1.0000x reference)
"""Trainium2 Bass kernel for nn_DynamicFeedForward (embedding-gather dot products).

Reference computation:
    part_weight = weight[mask]            # [b, s, 32, 512] gather
    out = einsum('bsh,bsmh->bsm', x, part_weight) + bias[mask]
    out = relu(out)

Strategy (data-parallel over tokens, 8 cores):
  - 8192 tokens sharded 1024/core, processed in groups of 128 (one token per
    SBUF partition).
  - Weight rows + bias are fetched with the GPSIMD dma_gather custom DMA:
    the host packs an augmented table [50000, 576] f32 (512 weight cols,
    bias at col 512, zero pad) so each gathered 2304B row carries its bias.
  - int16 index range trick: the gather base points at row 32768 and the
    host supplies int16(idx - 32768); the Q7's sign-extended address math
    then reaches rows 0..49999 while staying inside the table.
    HW constraints (measured): num_idxs % 128 == 0 and num_idxs <= 1024;
    TRAILING sign-negative indices are dropped as padding (mid-list ones
    gather normally), so the host permutes token-127's candidates to end
    every gather list with a non-negative index, with an exact host-side
    fixup for any slot that would still drop.
  - Gather list order puts gathered row i at partition i%128 = token, free
    block i//128 = candidate.
  - Compute per chunk: one batched DVE multiply [128, M_TILE, 512] (x read
    via a step-0 broadcast AP), then per-candidate reductions split between
    the scalar engine (Copy-activation with accum_out) and DVE tensor_reduce
    to balance engine busy time; gathered biases (col 512 of each row) are
    added per chunk, relu on the scalar engine, per-group store.
  - The kernel is DMA-bound: ~75.5 MB of gathered rows per core at
    ~326 GB/s effective.
"""

import numpy as np

N_CORES = 8
TOKENS = 4 * 2048
HIDDEN = 512
M = 32
VOCAB = 50000
P = 128
TOK_PER_CORE = TOKENS // N_CORES          # 1024
GROUPS = TOK_PER_CORE // P                # 8
M_TILE = 8                                # candidates per dma_gather chunk
NCHUNK = M // M_TILE                      # 4
ROW = 576                                 # augmented row elems (2304B, %256==0)
BASE = 32768                              # gather base row (int16 centering)
NIDX = P * M_TILE                         # 1024 indices per gather (%128, <=1024)

_cached = None


def _build_program(repeats=1, design="batched"):
    import concourse.bacc as bacc
    import concourse.mybir as mybir
    import concourse.tile as tile

    f32 = mybir.dt.float32
    i16 = mybir.dt.int16

    nc = bacc.Bacc(
        "TRN2",
        target_bir_lowering=False,
        debug=False,
        num_devices=N_CORES,
    )

    x_d = nc.dram_tensor("x", [TOK_PER_CORE, HIDDEN], f32, kind="ExternalInput")
    idx_d = nc.dram_tensor(
        "idx", [GROUPS * NCHUNK, P, NIDX // 16], i16, kind="ExternalInput"
    )
    w_d = nc.dram_tensor("w", [VOCAB, ROW], f32, kind="ExternalInput")
    out_d = nc.dram_tensor("out", [TOK_PER_CORE, M], f32, kind="ExternalOutput")

    with tile.TileContext(nc) as tc:
        with (
            tc.tile_pool(name="wg", bufs=5) as wpool,
            tc.tile_pool(name="xt", bufs=3) as xpool,
            tc.tile_pool(name="idxt", bufs=8) as ipool,
            tc.tile_pool(name="acct", bufs=4) as apool,
            tc.tile_pool(name="rest", bufs=4) as rpool,
            tc.tile_pool(name="relut", bufs=4) as relupool,
            tc.tile_pool(name="prod", bufs=2) as ppool,
            tc.tile_pool(name="dump", bufs=4) as dpool,
        ):
            for g in [g for _ in range(repeats) for g in range(GROUPS)]:
                tok = slice(g * P, (g + 1) * P)

                x_t = xpool.tile([P, HIDDEN], f32)
                nc.sync.dma_start(x_t[:], x_d[tok, :])

                acc_t = apool.tile([P, M], f32)
                res_t = rpool.tile([P, M], f32)
                for h in range(NCHUNK):
                    it = ipool.tile([P, NIDX // 16], i16)
                    nc.sync.dma_start(it[:], idx_d[g * NCHUNK + h, :, :])

                    w_t = wpool.tile([P, M_TILE * ROW], f32)
                    nc.gpsimd.dma_gather(
                        out_ap=w_t[:].rearrange("p (c e) -> p c e", e=ROW),
                        in_ap=w_d[BASE:, :],
                        idxs_ap=it[:],
                        num_idxs=NIDX,
                        num_idxs_reg=NIDX,
                        elem_size=ROW,
                    )
                    if design == "batched":
                        # one batched mul for the whole chunk: [128, M_TILE, 512]
                        prod = ppool.tile([P, M_TILE * HIDDEN], f32)
                        nc.vector.tensor_tensor(
                            out=prod[:].rearrange("p (c e) -> p c e", e=HIDDEN),
                            in0=w_t[:].rearrange("p (c e) -> p c e", e=ROW)[
                                :, :, :HIDDEN
                            ],
                            in1=x_t[:, None, :].to_broadcast([P, M_TILE, HIDDEN]),
                            op=mybir.AluOpType.mult,
                        )
                        # Reduce: ACT (copy-activation accumulate) carries most
                        # candidates; a few go to DVE tensor_reduce to balance
                        # engine busy time (ACT ~810ns/op vs DVE headroom).
                        n_dve = 1 if h % 2 == 0 else 2
                        for c in range(M_TILE):
                            mm = h * M_TILE + c
                            if c >= M_TILE - n_dve:
                                nc.vector.tensor_reduce(
                                    out=acc_t[:, mm : mm + 1],
                                    in_=prod[:, c * HIDDEN : (c + 1) * HIDDEN],
                                    axis=mybir.AxisListType.X,
                                    op=mybir.AluOpType.add,
                                )
                            else:
                                dump = dpool.tile([P, HIDDEN], f32)
                                nc.scalar.activation(
                                    out=dump[:],
                                    in_=prod[:, c * HIDDEN : (c + 1) * HIDDEN],
                                    func=mybir.ActivationFunctionType.Copy,
                                    accum_out=acc_t[:, mm : mm + 1],
                                )
                    else:
                        for c in range(M_TILE):
                            mm = h * M_TILE + c
                            prod = ppool.tile([P, HIDDEN], f32)
                            nc.vector.tensor_tensor(
                                out=prod[:],
                                in0=w_t[:, c * ROW : c * ROW + HIDDEN],
                                in1=x_t[:],
                                op=mybir.AluOpType.mult,
                            )
                            dump = dpool.tile([P, HIDDEN], f32)
                            nc.scalar.activation(
                                out=dump[:],
                                in_=prod[:],
                                func=mybir.ActivationFunctionType.Copy,
                                accum_out=acc_t[:, mm : mm + 1],
                            )
                    if True:
                        nc.vector.tensor_tensor(
                            out=res_t[:, h * M_TILE : (h + 1) * M_TILE],
                            in0=acc_t[:, h * M_TILE : (h + 1) * M_TILE],
                            in1=w_t[:].rearrange("p (c e) -> p c e", e=ROW)[
                                :, :, HIDDEN
                            ],
                            op=mybir.AluOpType.add,
                        )

                relu_t = relupool.tile([P, M], f32)
                nc.scalar.activation(
                    relu_t[:], res_t[:], mybir.ActivationFunctionType.Relu
                )
                nc.sync.dma_start(out_d[tok, :], relu_t[:])

    nc.compile()
    return nc


def _get_program():
    global _cached
    if _cached is None:
        _cached = _build_program()
    return _cached


def _plan_core(idx):
    """Plan one core's gather lists.

    idx: [TOK_PER_CORE, M] int64/int32 original indices.

    Returns (packed, cand_order, drops):
      packed: [GROUPS*NCHUNK, P, NIDX//16] int16 device index input
      cand_order: [TOK_PER_CORE, M] int; device res column k of token t holds
        candidate cand_order[t, k]
      drops: list of (t, k) device res slots that the HW will drop
        (trailing-negative padding rule) and the host must fix up
    """
    idx = idx.astype(np.int64)
    idx16 = (idx - BASE).astype(np.int16)  # [T, M]

    cand_order = np.tile(np.arange(M), (TOK_PER_CORE, 1))
    # For each group, permute the partition-127 token's candidates so each
    # chunk's final gather-list slot (token 127, block M_TILE-1) is >= 0.
    for g in range(GROUPS):
        t = g * P + (P - 1)
        high = np.flatnonzero(idx[t] >= BASE)
        low = np.flatnonzero(idx[t] < BASE)
        order = np.empty(M, np.int64)
        tail_slots = [h * M_TILE + (M_TILE - 1) for h in range(NCHUNK)]
        nh = min(len(high), NCHUNK)
        order[tail_slots[:nh]] = high[:nh]
        rest = np.concatenate([high[nh:], low])
        other_slots = [k for k in range(M) if k not in tail_slots[:nh]]
        order[other_slots] = rest
        cand_order[t] = order

    eff = np.take_along_axis(idx16, cand_order, axis=1)  # [T, M] device order

    packed = np.empty((GROUPS * NCHUNK, P, NIDX // 16), np.int16)
    drops = []
    for g in range(GROUPS):
        blk = eff[g * P : (g + 1) * P]  # [128, M]
        for h in range(NCHUNK):
            lst = blk[:, h * M_TILE : (h + 1) * M_TILE].T.reshape(NIDX).copy()
            if lst[NIDX - 1] < 0:
                # A list that ends sign-negative loses its tail (and an
                # all-negative list hard-faults the Q7) — force a valid
                # dummy index and let the host recompute that one slot.
                lst[NIDX - 1] = 0
                drops.append((g * P + (P - 1), h * M_TILE + (M_TILE - 1)))
            nonneg = np.flatnonzero(lst >= 0)
            last = nonneg[-1] if len(nonneg) else -1
            for i in range(last + 1, NIDX):
                p, c = i % P, i // P
                drops.append((g * P + p, h * M_TILE + c))
            wrapped = lst.reshape(NIDX // 16, 16).T  # [16, NIDX//16]
            packed[g * NCHUNK + h] = np.tile(wrapped, (8, 1))
    return packed, cand_order, drops


def kernel(input_value, mask_tensor, weight, bias):
    from concourse.bass_utils import run_bass_kernel_spmd

    x = np.ascontiguousarray(
        np.asarray(input_value).reshape(TOKENS, HIDDEN), dtype=np.float32
    )
    idx = np.asarray(mask_tensor).reshape(TOKENS, M)

    aug = np.zeros((VOCAB, ROW), np.float32)
    aug[:, :HIDDEN] = np.asarray(weight, np.float32)
    aug[:, HIDDEN] = np.asarray(bias, np.float32)

    nc = _get_program()

    in_maps = []
    plans = []
    for c in range(N_CORES):
        t = slice(c * TOK_PER_CORE, (c + 1) * TOK_PER_CORE)
        packed, cand_order, drops = _plan_core(idx[t])
        plans.append((cand_order, drops))
        in_maps.append({"x": x[t], "idx": packed, "w": aug})

    res = run_bass_kernel_spmd(nc, in_maps, core_ids=list(range(N_CORES)))
    kernel._last_results = res

    outs = []
    w32 = np.asarray(weight, np.float32)
    b32 = np.asarray(bias, np.float32)
    for c in range(N_CORES):
        dev = np.array(res.results[c]["out"])  # [T, M] in device cand order
        cand_order, drops = plans[c]
        t0 = c * TOK_PER_CORE
        for t_loc, k in drops:  # exact host fixup for HW-dropped tail slots
            cand = cand_order[t_loc, k]
            v = int(idx[t0 + t_loc, cand])
            dev[t_loc, k] = max(
                float(np.dot(x[t0 + t_loc], w32[v]) + b32[v]), 0.0
            )
        out = np.empty_like(dev)
        np.put_along_axis(out, cand_order, dev, axis=1)
        outs.append(out)

    out = np.concatenate(outs, axis=0)
    return out.reshape(mask_tensor.shape).astype(np.float32)


# revision 27
# speedup vs baseline: 1.1705x; 1.1705x over previous
"""Trainium2 Bass kernel for nn_DynamicFeedForward (embedding-gather dot products).

Reference computation:
    part_weight = weight[mask]            # [b, s, 32, 512] gather
    out = einsum('bsh,bsmh->bsm', x, part_weight) + bias[mask]
    out = relu(out)

Strategy (data-parallel over tokens, 8 cores):
  - 8192 tokens sharded 1024/core, processed in groups of 128 (one token per
    SBUF partition).
  - Weight rows + bias are fetched with the GPSIMD dma_gather custom DMA:
    the host packs an augmented table [50000, 576] f32 (512 weight cols,
    bias at col 512, zero pad) so each gathered 2304B row carries its bias.
  - int16 index range trick: the gather base points at row 32768 and the
    host supplies int16(idx - 32768); the Q7's sign-extended address math
    then reaches rows 0..49999 while staying inside the table.
    HW constraints (measured): num_idxs % 128 == 0 and num_idxs <= 1024;
    TRAILING sign-negative indices are dropped as padding (mid-list ones
    gather normally), so the host permutes token-127's candidates to end
    every gather list with a non-negative index, with an exact host-side
    fixup for any slot that would still drop.
  - Gather list order puts gathered row i at partition i%128 = token, free
    block i//128 = candidate.
  - Compute per chunk: one batched DVE multiply [128, M_TILE, 512] (x read
    via a step-0 broadcast AP), then per-candidate reductions split between
    the scalar engine (Copy-activation with accum_out) and DVE tensor_reduce
    to balance engine busy time; gathered biases (col 512 of each row) are
    added per chunk, relu on the scalar engine, per-group store.
  - The kernel is DMA-bound: ~75.5 MB of gathered rows per core at
    ~326 GB/s effective.
"""

import numpy as np

N_CORES = 8
TOKENS = 4 * 2048
HIDDEN = 512
M = 32
VOCAB = 50000
P = 128
TOK_PER_CORE = TOKENS // N_CORES          # 1024
GROUPS = TOK_PER_CORE // P                # 8
M_TILE = 8                                # candidates per dma_gather chunk
NCHUNK = M // M_TILE                      # 4
ROW = 576                                 # augmented row elems (2304B, %256==0)
BASE = 32768                              # gather base row (int16 centering)
NIDX = P * M_TILE                         # 1024 indices per gather (%128, <=1024)

_cached = None


def _build_program(repeats=1, design="batched"):
    import concourse.bacc as bacc
    import concourse.mybir as mybir
    import concourse.tile as tile

    f32 = mybir.dt.float32
    i16 = mybir.dt.int16

    nc = bacc.Bacc(
        "TRN2",
        target_bir_lowering=False,
        debug=False,
        num_devices=N_CORES,
    )

    x_d = nc.dram_tensor("x", [TOK_PER_CORE, HIDDEN], f32, kind="ExternalInput")
    idx_d = nc.dram_tensor(
        "idx", [GROUPS * NCHUNK, P, NIDX // 16], i16, kind="ExternalInput"
    )
    w_d = nc.dram_tensor("w", [VOCAB, ROW], f32, kind="ExternalInput")
    out_d = nc.dram_tensor("out", [TOK_PER_CORE, M], f32, kind="ExternalOutput")

    with tile.TileContext(nc) as tc:
        with (
            tc.tile_pool(name="wg", bufs=5) as wpool,
            tc.tile_pool(name="xt", bufs=3) as xpool,
            tc.tile_pool(name="idxt", bufs=8) as ipool,
            tc.tile_pool(name="acct", bufs=4) as apool,
            tc.tile_pool(name="rest", bufs=4) as rpool,
            tc.tile_pool(name="relut", bufs=4) as relupool,
            tc.tile_pool(name="prod", bufs=2) as ppool,
            tc.tile_pool(name="dump", bufs=4) as dpool,
        ):
            for g in [g for _ in range(repeats) for g in range(GROUPS)]:
                tok = slice(g * P, (g + 1) * P)

                x_t = xpool.tile([P, HIDDEN], f32)
                nc.sync.dma_start(x_t[:], x_d[tok, :])

                acc_t = apool.tile([P, M], f32)
                res_t = rpool.tile([P, M], f32)
                for h in range(NCHUNK):
                    it = ipool.tile([P, NIDX // 16], i16)
                    nc.sync.dma_start(it[:], idx_d[g * NCHUNK + h, :, :])

                    w_t = wpool.tile([P, M_TILE * ROW], f32)
                    nc.gpsimd.dma_gather(
                        out_ap=w_t[:].rearrange("p (c e) -> p c e", e=ROW),
                        in_ap=w_d[BASE:, :],
                        idxs_ap=it[:],
                        num_idxs=NIDX,
                        num_idxs_reg=NIDX,
                        elem_size=ROW,
                    )
                    if design == "batched":
                        # one batched mul for the whole chunk: [128, M_TILE, 512]
                        prod = ppool.tile([P, M_TILE * HIDDEN], f32)
                        nc.vector.tensor_tensor(
                            out=prod[:].rearrange("p (c e) -> p c e", e=HIDDEN),
                            in0=w_t[:].rearrange("p (c e) -> p c e", e=ROW)[
                                :, :, :HIDDEN
                            ],
                            in1=x_t[:, None, :].to_broadcast([P, M_TILE, HIDDEN]),
                            op=mybir.AluOpType.mult,
                        )
                        # Reduce: ACT (copy-activation accumulate) carries most
                        # candidates; a few go to DVE tensor_reduce to balance
                        # engine busy time (ACT ~810ns/op vs DVE headroom).
                        n_dve = 1 if h % 2 == 0 else 2
                        for c in range(M_TILE):
                            mm = h * M_TILE + c
                            if c >= M_TILE - n_dve:
                                nc.vector.tensor_reduce(
                                    out=acc_t[:, mm : mm + 1],
                                    in_=prod[:, c * HIDDEN : (c + 1) * HIDDEN],
                                    axis=mybir.AxisListType.X,
                                    op=mybir.AluOpType.add,
                                )
                            else:
                                dump = dpool.tile([P, HIDDEN], f32)
                                nc.scalar.activation(
                                    out=dump[:],
                                    in_=prod[:, c * HIDDEN : (c + 1) * HIDDEN],
                                    func=mybir.ActivationFunctionType.Copy,
                                    accum_out=acc_t[:, mm : mm + 1],
                                )
                    else:
                        for c in range(M_TILE):
                            mm = h * M_TILE + c
                            prod = ppool.tile([P, HIDDEN], f32)
                            nc.vector.tensor_tensor(
                                out=prod[:],
                                in0=w_t[:, c * ROW : c * ROW + HIDDEN],
                                in1=x_t[:],
                                op=mybir.AluOpType.mult,
                            )
                            dump = dpool.tile([P, HIDDEN], f32)
                            nc.scalar.activation(
                                out=dump[:],
                                in_=prod[:],
                                func=mybir.ActivationFunctionType.Copy,
                                accum_out=acc_t[:, mm : mm + 1],
                            )
                    # add the gathered biases (col HIDDEN of each block)
                    nc.vector.tensor_tensor(
                        out=res_t[:, h * M_TILE : (h + 1) * M_TILE],
                        in0=acc_t[:, h * M_TILE : (h + 1) * M_TILE],
                        in1=w_t[:].rearrange("p (c e) -> p c e", e=ROW)[
                            :, :, HIDDEN
                        ],
                        op=mybir.AluOpType.add,
                    )

                relu_t = relupool.tile([P, M], f32)
                nc.scalar.activation(
                    relu_t[:], res_t[:], mybir.ActivationFunctionType.Relu
                )
                nc.sync.dma_start(out_d[tok, :], relu_t[:])

    nc.compile()
    return nc


def _get_program():
    global _cached
    if _cached is None:
        _cached = _build_program()
    return _cached


def _plan_core(idx):
    """Plan one core's gather lists.

    idx: [TOK_PER_CORE, M] int64/int32 original indices.

    Returns (packed, cand_order, drops):
      packed: [GROUPS*NCHUNK, P, NIDX//16] int16 device index input
      cand_order: [TOK_PER_CORE, M] int; device res column k of token t holds
        candidate cand_order[t, k]
      drops: list of (t, k) device res slots that the HW will drop
        (trailing-negative padding rule) and the host must fix up
    """
    idx = idx.astype(np.int64)
    idx16 = (idx - BASE).astype(np.int16)  # [T, M]

    cand_order = np.tile(np.arange(M), (TOK_PER_CORE, 1))
    # For each group, permute the partition-127 token's candidates so each
    # chunk's final gather-list slot (token 127, block M_TILE-1) is >= 0.
    for g in range(GROUPS):
        t = g * P + (P - 1)
        high = np.flatnonzero(idx[t] >= BASE)
        low = np.flatnonzero(idx[t] < BASE)
        order = np.empty(M, np.int64)
        tail_slots = [h * M_TILE + (M_TILE - 1) for h in range(NCHUNK)]
        nh = min(len(high), NCHUNK)
        order[tail_slots[:nh]] = high[:nh]
        rest = np.concatenate([high[nh:], low])
        other_slots = [k for k in range(M) if k not in tail_slots[:nh]]
        order[other_slots] = rest
        cand_order[t] = order

    eff = np.take_along_axis(idx16, cand_order, axis=1)  # [T, M] device order

    packed = np.empty((GROUPS * NCHUNK, P, NIDX // 16), np.int16)
    drops = []
    for g in range(GROUPS):
        blk = eff[g * P : (g + 1) * P]  # [128, M]
        for h in range(NCHUNK):
            lst = blk[:, h * M_TILE : (h + 1) * M_TILE].T.reshape(NIDX).copy()
            if lst[NIDX - 1] < 0:
                # A list that ends sign-negative loses its tail (and an
                # all-negative list hard-faults the Q7) — force a valid
                # dummy index and let the host recompute that one slot.
                lst[NIDX - 1] = 0
                drops.append((g * P + (P - 1), h * M_TILE + (M_TILE - 1)))
            nonneg = np.flatnonzero(lst >= 0)
            last = nonneg[-1] if len(nonneg) else -1
            for i in range(last + 1, NIDX):
                p, c = i % P, i // P
                drops.append((g * P + p, h * M_TILE + c))
            wrapped = lst.reshape(NIDX // 16, 16).T  # [16, NIDX//16]
            packed[g * NCHUNK + h] = np.tile(wrapped, (8, 1))
    return packed, cand_order, drops


def kernel(input_value, mask_tensor, weight, bias):
    from concourse.bass_utils import run_bass_kernel_spmd

    x = np.ascontiguousarray(
        np.asarray(input_value).reshape(TOKENS, HIDDEN), dtype=np.float32
    )
    idx = np.asarray(mask_tensor).reshape(TOKENS, M)

    aug = np.zeros((VOCAB, ROW), np.float32)
    aug[:, :HIDDEN] = np.asarray(weight, np.float32)
    aug[:, HIDDEN] = np.asarray(bias, np.float32)

    nc = _get_program()

    in_maps = []
    plans = []
    for c in range(N_CORES):
        t = slice(c * TOK_PER_CORE, (c + 1) * TOK_PER_CORE)
        packed, cand_order, drops = _plan_core(idx[t])
        plans.append((cand_order, drops))
        in_maps.append({"x": x[t], "idx": packed, "w": aug})

    res = run_bass_kernel_spmd(nc, in_maps, core_ids=list(range(N_CORES)))
    kernel._last_results = res

    outs = []
    w32 = np.asarray(weight, np.float32)
    b32 = np.asarray(bias, np.float32)
    for c in range(N_CORES):
        dev = np.array(res.results[c]["out"])  # [T, M] in device cand order
        cand_order, drops = plans[c]
        t0 = c * TOK_PER_CORE
        for t_loc, k in drops:  # exact host fixup for HW-dropped tail slots
            cand = cand_order[t_loc, k]
            v = int(idx[t0 + t_loc, cand])
            dev[t_loc, k] = max(
                float(np.dot(x[t0 + t_loc], w32[v]) + b32[v]), 0.0
            )
        out = np.empty_like(dev)
        np.put_along_axis(out, cand_order, dev, axis=1)
        outs.append(out)

    out = np.concatenate(outs, axis=0)
    return out.reshape(mask_tensor.shape).astype(np.float32)
